# revision 1
# baseline (speedup 1.0000x reference)
"""Baseline kernel (restored from the originally staged version)."""

import numpy as np

B, S, D, H, HD = 2, 2048, 1024, 16, 64
N_CORES = 8
SCALE = HD ** (-0.5)
BS = B * S               # 4096
SC = 512                 # qkv-phase s-chunk (8 chunks)
QC = 512                 # attention q-chunk
NKC = S // 128           # 16 k-chunks per batch
DC = D // 128            # 8 contraction chunks

_cache = {}


def _build():
    import concourse.bass as bass
    import concourse.mybir as mybir
    import concourse.tile as tile
    from concourse import bacc

    F32 = mybir.dt.float32
    F32R = mybir.dt.float32r
    BF16 = mybir.dt.bfloat16
    F16 = mybir.dt.float16
    AF = mybir.ActivationFunctionType

    nc = bacc.Bacc("TRN2", target_bir_lowering=False, debug=False,
                   num_devices=N_CORES)
    xT_d = nc.dram_tensor("xT", (D, BS), BF16, kind="ExternalInput").ap()
    wqkvT_d = nc.dram_tensor("wqkvT", (D, 384), BF16, kind="ExternalInput").ap()
    woutT_d = nc.dram_tensor("woutT", (128, D), BF16, kind="ExternalInput").ap()
    out_d = nc.dram_tensor("out", (BS, D), F16, kind="ExternalOutput").ap()

    with tile.TileContext(nc) as tc:
        with tc.tile_pool(name="persist", bufs=1) as persist, \
             tc.tile_pool(name="xin", bufs=3) as xin, \
             tc.tile_pool(name="epool", bufs=4) as epool, \
             tc.tile_pool(name="work", bufs=3) as work, \
             tc.tile_pool(name="ps_sc", bufs=2, space="PSUM") as ps_sc, \
             tc.tile_pool(name="pacc", bufs=4, space="PSUM") as pacc:

            # ---- persistent tiles ----
            wqkvT = persist.tile([128, DC, 384], BF16, tag="wqkvT")
            nc.sync.dma_start(wqkvT[:], wqkvT_d.rearrange("(po pi) e -> pi po e", pi=128))
            woutT = persist.tile([128, D], BF16, tag="woutT")
            nc.sync.dma_start(woutT[:], woutT_d)

            ident = persist.tile([128, 128], F32, tag="ident")
            from concourse.masks import make_identity
            make_identity(nc, ident[:])

            QT = persist.tile([128, BS], BF16, tag="QT")
            KT = persist.tile([128, BS], BF16, tag="KT")
            VT = persist.tile([128, BS], F32, tag="VT")
            vaug = [[persist.tile([128, NKC, 128], BF16, tag=f"vaug{b}{h}",
                                  name=f"vaug{b}{h}")
                     for h in range(2)] for b in range(B)]
            const_f32 = persist.tile([128, NKC * 64], F32, tag="const_f32")
            nc.gpsimd.memset(const_f32[:], 1.0)
            inv2 = persist.tile([128, 128], F32R, tag="inv2")
            inv2_f32 = persist.tile([128, 128], F32, tag="inv2_f32")
            nc.gpsimd.memset(inv2_f32[:], 0.0)
            nc.gpsimd.memset(inv2_f32[64:128, 0:64], 1.0 / 64.0)
            nc.gpsimd.memset(inv2_f32[0:64, 64:128], 1.0 / 64.0)
            nc.vector.tensor_copy(inv2[:], inv2_f32[:])
            ones_3d = const_f32[:].rearrange("p (a b) -> p a b", b=64)
            for b in range(B):
                nc.vector.tensor_copy(vaug[b][0][:, :, 64:128], ones_3d)
                nc.vector.tensor_copy(vaug[b][1][:, :, 0:64], ones_3d)

            xts = {}

            def emit_xt_dma(s):
                xt = xin.tile([128, DC, SC], BF16, tag="xt", name="xt")
                nc.sync.dma_start(
                    xt[:], xT_d[:, s * SC:(s + 1) * SC]
                    .rearrange("(po pi) s -> pi po s", pi=128))
                xts[s] = xt

            def emit_qkv_part(s, e):
                dst = (QT, KT, VT)[e]
                ps = pacc.tile([128, SC], F32, tag="pacc", name="qkv_ps")
                for d in range(DC):
                    nc.tensor.matmul(
                        ps[:], lhsT=wqkvT[:, d, 128 * e:128 * (e + 1)],
                        rhs=xts[s][:, d, :], start=(d == 0), stop=(d == DC - 1))
                nc.vector.tensor_copy(dst[:, s * SC:(s + 1) * SC], ps[:])

            def emit_qkv(s):
                emit_xt_dma(s)
                for e in range(3):
                    emit_qkv_part(s, e)

            def emit_vtrans(j):
                b, k = divmod(j, NKC)
                ps = pacc.tile([128, SC], F32, tag="pacc")
                pt = ps[:, 0:128]
                nc.tensor.transpose(pt, VT[:, j * 128:(j + 1) * 128], ident[:])
                nc.vector.tensor_copy(vaug[b][0][:, k, 0:64], pt[:, 0:64])
                nc.vector.tensor_copy(vaug[b][1][:, k, 64:128], pt[:, 64:128])

            def emit_finish_stage(st, stage):
                if st is None:
                    return
                if stage == 0:
                    st["invd"] = work.tile([128, QC], F32, tag="invd", name="invd")
                    st["pbc"] = pacc.tile([128, SC], F32, tag="pacc", name="pbc")
                    nc.tensor.matmul(st["pbc"][:], lhsT=inv2[64:128, :],
                                     rhs=st["odA"][64:128, :],
                                     start=True, stop=False)
                    nc.tensor.matmul(st["pbc"][:], lhsT=inv2[0:64, :],
                                     rhs=st["odB"][0:64, :],
                                     start=False, stop=True)
                    nc.vector.reciprocal(st["invd"][:], st["pbc"][:])
                elif stage == 1:
                    st["ot"] = work.tile([128, QC], BF16, tag="ot", name="ot")
                    nc.vector.tensor_mul(out=st["ot"][0:64, :],
                                         in0=st["odA"][0:64, :].bitcast(F32),
                                         in1=st["invd"][0:64, :])
                    nc.vector.tensor_mul(out=st["ot"][64:128, :],
                                         in0=st["odB"][64:128, :].bitcast(F32),
                                         in1=st["invd"][64:128, :])
                else:
                    j = stage - 2
                    osb = work.tile([128, D], F16, tag="osb")
                    for e in range(D // SC):
                        po = pacc.tile([128, SC], F32, tag="pacc")
                        nc.tensor.matmul(
                            po[:], lhsT=st["ot"][:, j * 128:(j + 1) * 128],
                            rhs=woutT[:, e * SC:(e + 1) * SC],
                            start=True, stop=True)
                        nc.vector.tensor_copy(osb[:, e * SC:(e + 1) * SC], po[:])
                    row = st["q0"] + j * 128
                    nc.sync.dma_start(out_d[row:row + 128, :], osb[:])

            FIN_AT = {4: 0, 6: 1, 8: 2, 10: 3}

            def emit_attn(b, q, prev, filler=None):
                filler = filler or {}
                q0 = b * S + q * QC
                psA = pacc.tile([128, SC], F32, tag="pacc")
                psB = pacc.tile([128, SC], F32, tag="pacc")
                for k in range(NKC):
                    kcol = b * S + k * 128
                    pss = ps_sc.tile([128, 2 * QC], F32, tag="scores")
                    nc.tensor.matmul(
                        pss[:, 0:QC], lhsT=KT[0:64, kcol:kcol + 128],
                        rhs=QT[0:64, q0:q0 + QC], start=True, stop=True)
                    nc.tensor.matmul(
                        pss[:, QC:2 * QC], lhsT=KT[64:128, kcol:kcol + 128],
                        rhs=QT[64:128, q0:q0 + QC], start=True, stop=True)
                    eb = epool.tile([128, 2 * QC], BF16, tag="eb")
                    nc.scalar.activation(eb[:], pss[:], AF.Exp, scale=float(SCALE))
                    nc.tensor.matmul(psA[:], lhsT=vaug[b][0][:, k, :],
                                     rhs=eb[:, 0:QC],
                                     start=(k == 0), stop=(k == NKC - 1))
                    nc.tensor.matmul(psB[:], lhsT=vaug[b][1][:, k, :],
                                     rhs=eb[:, QC:2 * QC],
                                     start=(k == 0), stop=(k == NKC - 1))
                    if k in FIN_AT:
                        emit_finish_stage(prev, FIN_AT[k])
                    for fn in filler.get(k, ()):
                        fn()
                odA = work.tile([128, QC], F32R, tag="odA")
                odB = work.tile([128, QC], F32R, tag="odB")
                nc.vector.tensor_copy(odA[:], psA[:])
                nc.vector.tensor_copy(odB[:], psB[:])
                emit_finish_stage(prev, 4)
                emit_finish_stage(prev, 5)
                return {"q0": q0, "odA": odA, "odB": odB}

            for s in range(4):
                emit_qkv(s)
                for j in range(4 * s, 4 * s + 4):
                    emit_vtrans(j)
            prev = None
            for q in range(4):
                prev = emit_attn(0, q, prev)
                s = 4 + q
                emit_qkv(s)
                for j in range(4 * s, 4 * s + 4):
                    emit_vtrans(j)
            for q in range(4):
                prev = emit_attn(1, q, prev)
            for stage in range(4):
                emit_finish_stage(prev, stage)
            emit_finish_stage(prev, 4)
            emit_finish_stage(prev, 5)

    nc.compile()
    return nc


def _get_nc():
    if "nc" not in _cache:
        _cache["nc"] = _build()
    return _cache["nc"]


def _prep_inputs(x, w_qkv, w_out):
    import ml_dtypes
    bf16 = ml_dtypes.bfloat16
    x = np.asarray(x, dtype=np.float32)
    w_qkv = np.asarray(w_qkv, dtype=np.float32)
    w_out = np.asarray(w_out, dtype=np.float32)
    xT = np.ascontiguousarray(x.reshape(BS, D).T.astype(bf16))
    in_maps = []
    for c in range(N_CORES):
        wq = w_qkv[D + 128 * c: D + 128 * (c + 1)]
        wk = w_qkv[2 * D + 128 * c: 2 * D + 128 * (c + 1)]
        wv = w_qkv[128 * c: 128 * (c + 1)]
        wqkvT = np.ascontiguousarray(
            np.concatenate([wq, wk, wv], axis=0).T.astype(bf16))
        woutT = np.ascontiguousarray(
            w_out[:, 128 * c:128 * (c + 1)].T.astype(bf16))
        in_maps.append({"xT": xT, "wqkvT": wqkvT, "woutT": woutT})
    return in_maps


def kernel(x, w_qkv, w_out, b_out):
    from concourse.bass_utils import run_bass_kernel_spmd

    nc = _get_nc()
    in_maps = _prep_inputs(x, w_qkv, w_out)
    b_out = np.asarray(b_out, dtype=np.float32)
    res = run_bass_kernel_spmd(nc, in_maps, core_ids=list(range(N_CORES)))
    acc = np.zeros((BS, D), np.float32)
    for c in range(N_CORES):
        acc += res.results[c]["out"].astype(np.float32)
    acc = acc + b_out[None, :]
    return acc.reshape(B, S, D)



# revision 5
# speedup vs baseline: 1.0039x; 1.0039x over previous
"""Multi-head attention forward, sharded 2-heads-per-core over 8 cores.

V2: dedicated PSUM pools (scores / accumulators / misc) so the attnV
accumulators no longer starve the transient tiles, fast approximate
reciprocal for the softmax denominators, bf16 finish path.
"""

import numpy as np

B, S, D, H, HD = 2, 2048, 1024, 16, 64
N_CORES = 8
SCALE = HD ** (-0.5)
BS = B * S               # 4096
SC = 512                 # qkv-phase s-chunk (8 chunks)
QC = 512                 # attention q-chunk
NKC = S // 128           # 16 k-chunks per batch
DC = D // 128            # 8 contraction chunks

_cache = {}


def _build():
    import concourse.bass as bass
    import concourse.mybir as mybir
    import concourse.tile as tile
    from concourse import bacc

    F32 = mybir.dt.float32
    F32R = mybir.dt.float32r
    BF16 = mybir.dt.bfloat16
    F16 = mybir.dt.float16
    AF = mybir.ActivationFunctionType

    nc = bacc.Bacc("TRN2", target_bir_lowering=False, debug=False,
                   num_devices=N_CORES)
    xT_d = nc.dram_tensor("xT", (D, BS), BF16, kind="ExternalInput").ap()
    wqkvT_d = nc.dram_tensor("wqkvT", (D, 384), BF16, kind="ExternalInput").ap()
    woutT_d = nc.dram_tensor("woutT", (128, D), BF16, kind="ExternalInput").ap()
    out_d = nc.dram_tensor("out", (BS, D), F16, kind="ExternalOutput").ap()

    with tile.TileContext(nc) as tc:
        with tc.tile_pool(name="persist", bufs=1) as persist, \
             tc.tile_pool(name="xin", bufs=3) as xin, \
             tc.tile_pool(name="epool", bufs=4) as epool, \
             tc.tile_pool(name="work", bufs=3) as work, \
             tc.tile_pool(name="ps_sc", bufs=2, space="PSUM") as ps_sc, \
             tc.tile_pool(name="pacc", bufs=4, space="PSUM") as pacc:

            # ---- persistent tiles ----
            wqkvT = persist.tile([128, DC, 384], BF16, tag="wqkvT")
            nc.sync.dma_start(wqkvT[:], wqkvT_d.rearrange("(po pi) e -> pi po e", pi=128))
            woutT = persist.tile([128, D], BF16, tag="woutT")
            nc.sync.dma_start(woutT[:], woutT_d)

            ident = persist.tile([128, 128], F32, tag="ident")
            from concourse.masks import make_identity
            make_identity(nc, ident[:])

            QT = persist.tile([128, BS], BF16, tag="QT")
            KT = persist.tile([128, BS], BF16, tag="KT")
            VT = persist.tile([128, BS], F32, tag="VT")
            vaug = [[persist.tile([128, NKC, 128], BF16, tag=f"vaug{b}{h}",
                                  name=f"vaug{b}{h}")
                     for h in range(2)] for b in range(B)]
            const_f32 = persist.tile([128, NKC * 64], F32, tag="const_f32")
            nc.gpsimd.memset(const_f32[:], 1.0)
            inv2 = persist.tile([128, 128], F32R, tag="inv2")
            inv2_f32 = persist.tile([128, 128], F32, tag="inv2_f32")
            nc.gpsimd.memset(inv2_f32[:], 0.0)
            nc.gpsimd.memset(inv2_f32[64:128, 0:64], 1.0 / 64.0)
            nc.gpsimd.memset(inv2_f32[0:64, 64:128], 1.0 / 64.0)
            nc.vector.tensor_copy(inv2[:], inv2_f32[:])
            ones_3d = const_f32[:].rearrange("p (a b) -> p a b", b=64)
            for b in range(B):
                nc.vector.tensor_copy(vaug[b][0][:, :, 64:128], ones_3d)
                nc.vector.tensor_copy(vaug[b][1][:, :, 0:64], ones_3d)

            xts = {}

            def emit_xt_dma(s):
                xt = xin.tile([128, DC, SC], BF16, tag="xt", name="xt")
                nc.sync.dma_start(
                    xt[:], xT_d[:, s * SC:(s + 1) * SC]
                    .rearrange("(po pi) s -> pi po s", pi=128))
                xts[s] = xt

            def emit_qkv_part(s, e):
                dst = (QT, KT, VT)[e]
                ps = pacc.tile([128, SC], F32, tag="pacc", name="qkv_ps")
                for d in range(DC):
                    nc.tensor.matmul(
                        ps[:], lhsT=wqkvT[:, d, 128 * e:128 * (e + 1)],
                        rhs=xts[s][:, d, :], start=(d == 0), stop=(d == DC - 1))
                nc.vector.tensor_copy(dst[:, s * SC:(s + 1) * SC], ps[:])

            def emit_qkv(s):
                emit_xt_dma(s)
                for e in range(3):
                    emit_qkv_part(s, e)

            def emit_vtrans(j):
                b, k = divmod(j, NKC)
                ps = pacc.tile([128, SC], F32, tag="pacc", name="vt_ps")
                pt = ps[:, 0:128]
                nc.tensor.transpose(pt, VT[:, j * 128:(j + 1) * 128], ident[:])
                nc.vector.tensor_copy(vaug[b][0][:, k, 0:64], pt[:, 0:64])
                nc.vector.tensor_copy(vaug[b][1][:, k, 64:128], pt[:, 64:128])

            def emit_finish_stage(st, stage):
                if st is None:
                    return
                if stage == 0:
                    st["invd"] = work.tile([128, QC], F32, tag="invd", name="invd")
                    st["pbc"] = pacc.tile([128, SC], F32, tag="pacc", name="pbc")
                    nc.tensor.matmul(st["pbc"][:], lhsT=inv2[64:128, :],
                                     rhs=st["odA"][64:128, :],
                                     start=True, stop=False)
                    nc.tensor.matmul(st["pbc"][:], lhsT=inv2[0:64, :],
                                     rhs=st["odB"][0:64, :],
                                     start=False, stop=True)
                    nc.vector.reciprocal(st["invd"][:], st["pbc"][:])
                elif stage == 1:
                    st["ot"] = work.tile([128, QC], BF16, tag="ot", name="ot")
                    nc.vector.tensor_mul(out=st["ot"][0:64, :],
                                         in0=st["odA"][0:64, :].bitcast(F32),
                                         in1=st["invd"][0:64, :])
                    nc.vector.tensor_mul(out=st["ot"][64:128, :],
                                         in0=st["odB"][64:128, :].bitcast(F32),
                                         in1=st["invd"][64:128, :])
                else:
                    j = stage - 2
                    osb = work.tile([128, D], F16, tag="osb")
                    for e in range(D // SC):
                        po = pacc.tile([128, SC], F32, tag="pacc", name="po")
                        nc.tensor.matmul(
                            po[:], lhsT=st["ot"][:, j * 128:(j + 1) * 128],
                            rhs=woutT[:, e * SC:(e + 1) * SC],
                            start=True, stop=True)
                        nc.vector.tensor_copy(osb[:, e * SC:(e + 1) * SC], po[:])
                    row = st["q0"] + j * 128
                    nc.sync.dma_start(out_d[row:row + 128, :], osb[:])

            FIN_AT = {4: 0, 6: 1, 8: 2, 10: 3}

            def emit_attn(b, q, prev, filler=None):
                filler = filler or {}
                q0 = b * S + q * QC
                psA = pacc.tile([128, SC], F32, tag="pacc", name="psA")
                psB = pacc.tile([128, SC], F32, tag="pacc", name="psB")
                for k in range(NKC):
                    kcol = b * S + k * 128
                    pss = ps_sc.tile([128, 2 * QC], F32, tag="scores")
                    nc.tensor.matmul(
                        pss[:, 0:QC], lhsT=KT[0:64, kcol:kcol + 128],
                        rhs=QT[0:64, q0:q0 + QC], start=True, stop=True)
                    nc.tensor.matmul(
                        pss[:, QC:2 * QC], lhsT=KT[64:128, kcol:kcol + 128],
                        rhs=QT[64:128, q0:q0 + QC], start=True, stop=True)
                    eb = epool.tile([128, 2 * QC], BF16, tag="eb")
                    nc.scalar.activation(eb[:], pss[:], AF.Exp, scale=float(SCALE))
                    nc.tensor.matmul(psA[:], lhsT=vaug[b][0][:, k, :],
                                     rhs=eb[:, 0:QC],
                                     start=(k == 0), stop=(k == NKC - 1))
                    nc.tensor.matmul(psB[:], lhsT=vaug[b][1][:, k, :],
                                     rhs=eb[:, QC:2 * QC],
                                     start=(k == 0), stop=(k == NKC - 1))
                    if k in FIN_AT:
                        emit_finish_stage(prev, FIN_AT[k])
                    for fn in filler.get(k, ()):
                        fn()
                odA = work.tile([128, QC], F32R, tag="odA", name="odA")
                odB = work.tile([128, QC], F32R, tag="odB", name="odB")
                nc.vector.tensor_copy(odA[:], psA[:])
                nc.vector.tensor_copy(odB[:], psB[:])
                emit_finish_stage(prev, 4)
                emit_finish_stage(prev, 5)
                return {"q0": q0, "odA": odA, "odB": odB}

            for s in range(4):
                emit_qkv(s)
                for j in range(4 * s, 4 * s + 4):
                    emit_vtrans(j)
            prev = None
            for q in range(4):
                prev = emit_attn(0, q, prev)
                s = 4 + q
                emit_qkv(s)
                for j in range(4 * s, 4 * s + 4):
                    emit_vtrans(j)
            for q in range(4):
                prev = emit_attn(1, q, prev)
            for stage in range(4):
                emit_finish_stage(prev, stage)
            emit_finish_stage(prev, 4)
            emit_finish_stage(prev, 5)

    nc.compile()
    return nc


def _get_nc():
    if "nc" not in _cache:
        _cache["nc"] = _build()
    return _cache["nc"]


def _prep_inputs(x, w_qkv, w_out):
    import ml_dtypes
    bf16 = ml_dtypes.bfloat16
    x = np.asarray(x, dtype=np.float32)
    w_qkv = np.asarray(w_qkv, dtype=np.float32)
    w_out = np.asarray(w_out, dtype=np.float32)
    xT = np.ascontiguousarray(x.reshape(BS, D).T.astype(bf16))
    in_maps = []
    for c in range(N_CORES):
        wq = w_qkv[D + 128 * c: D + 128 * (c + 1)]
        wk = w_qkv[2 * D + 128 * c: 2 * D + 128 * (c + 1)]
        wv = w_qkv[128 * c: 128 * (c + 1)]
        wqkvT = np.ascontiguousarray(
            np.concatenate([wq, wk, wv], axis=0).T.astype(bf16))
        woutT = np.ascontiguousarray(
            w_out[:, 128 * c:128 * (c + 1)].T.astype(bf16))
        in_maps.append({"xT": xT, "wqkvT": wqkvT, "woutT": woutT})
    return in_maps


def kernel(x, w_qkv, w_out, b_out):
    from concourse.bass_utils import run_bass_kernel_spmd

    nc = _get_nc()
    in_maps = _prep_inputs(x, w_qkv, w_out)
    b_out = np.asarray(b_out, dtype=np.float32)
    res = run_bass_kernel_spmd(nc, in_maps, core_ids=list(range(N_CORES)))
    acc = np.zeros((BS, D), np.float32)
    for c in range(N_CORES):
        acc += res.results[c]["out"].astype(np.float32)
    acc = acc + b_out[None, :]
    return acc.reshape(B, S, D)


# revision 8
# speedup vs baseline: 1.1181x; 1.1138x over previous
"""Multi-head attention forward, sharded 2-heads-per-core over 8 cores.

V2: dedicated PSUM pools (scores / accumulators / misc) so the attnV
accumulators no longer starve the transient tiles, fast approximate
reciprocal for the softmax denominators, bf16 finish path.
"""

import numpy as np

B, S, D, H, HD = 2, 2048, 1024, 16, 64
N_CORES = 8
SCALE = HD ** (-0.5)
BS = B * S               # 4096
SC = 512                 # qkv-phase s-chunk (8 chunks)
QC = 512                 # attention q-chunk
NKC = S // 128           # 16 k-chunks per batch
DC = D // 128            # 8 contraction chunks

_cache = {}


def _build():
    import concourse.bass as bass
    import concourse.mybir as mybir
    import concourse.tile as tile
    from concourse import bacc

    F32 = mybir.dt.float32
    F32R = mybir.dt.float32r
    BF16 = mybir.dt.bfloat16
    F16 = mybir.dt.float16
    AF = mybir.ActivationFunctionType

    nc = bacc.Bacc("TRN2", target_bir_lowering=False, debug=False,
                   num_devices=N_CORES)
    xT_d = nc.dram_tensor("xT", (D, BS), BF16, kind="ExternalInput").ap()
    wqkvT_d = nc.dram_tensor("wqkvT", (D, 384), BF16, kind="ExternalInput").ap()
    woutT_d = nc.dram_tensor("woutT", (128, D), BF16, kind="ExternalInput").ap()
    out_d = nc.dram_tensor("out", (BS, D), F16, kind="ExternalOutput").ap()

    with tile.TileContext(nc) as tc:
        with tc.tile_pool(name="persist", bufs=1) as persist, \
             tc.tile_pool(name="xin", bufs=3) as xin, \
             tc.tile_pool(name="epool", bufs=6) as epool, \
             tc.tile_pool(name="work", bufs=3) as work, \
             tc.tile_pool(name="ps_sc", bufs=2, space="PSUM") as ps_sc, \
             tc.tile_pool(name="pacc", bufs=4, space="PSUM") as pacc:

            # ---- persistent tiles ----
            wqkvT = persist.tile([128, DC, 384], BF16, tag="wqkvT")
            nc.sync.dma_start(wqkvT[:], wqkvT_d.rearrange("(po pi) e -> pi po e", pi=128))
            woutT = persist.tile([128, D], BF16, tag="woutT")
            nc.sync.dma_start(woutT[:], woutT_d)

            ident = persist.tile([128, 128], F32, tag="ident")
            from concourse.masks import make_identity
            make_identity(nc, ident[:])

            QT = persist.tile([128, BS], BF16, tag="QT")
            KT = persist.tile([128, BS], BF16, tag="KT")
            VT = persist.tile([128, BS], F32, tag="VT")
            vaug = [[persist.tile([128, NKC, 128], BF16, tag=f"vaug{b}{h}",
                                  name=f"vaug{b}{h}")
                     for h in range(2)] for b in range(B)]
            const_f32 = persist.tile([128, NKC * 64], F32, tag="const_f32")
            nc.gpsimd.memset(const_f32[:], 1.0)
            inv2 = persist.tile([128, 128], F32R, tag="inv2")
            inv2_f32 = persist.tile([128, 128], F32, tag="inv2_f32")
            nc.gpsimd.memset(inv2_f32[:], 0.0)
            nc.gpsimd.memset(inv2_f32[64:128, 0:64], 1.0 / 64.0)
            nc.gpsimd.memset(inv2_f32[0:64, 64:128], 1.0 / 64.0)
            nc.vector.tensor_copy(inv2[:], inv2_f32[:])
            ones_3d = const_f32[:].rearrange("p (a b) -> p a b", b=64)
            for b in range(B):
                nc.vector.tensor_copy(vaug[b][0][:, :, 64:128], ones_3d)
                nc.vector.tensor_copy(vaug[b][1][:, :, 0:64], ones_3d)

            xts = {}

            def emit_xt_dma(s):
                xt = xin.tile([128, DC, SC], BF16, tag="xt", name="xt")
                nc.sync.dma_start(
                    xt[:], xT_d[:, s * SC:(s + 1) * SC]
                    .rearrange("(po pi) s -> pi po s", pi=128))
                xts[s] = xt

            def emit_qkv_part(s, e):
                dst = (QT, KT, VT)[e]
                ps = pacc.tile([128, SC], F32, tag="pacc", name="qkv_ps")
                for d in range(DC):
                    nc.tensor.matmul(
                        ps[:], lhsT=wqkvT[:, d, 128 * e:128 * (e + 1)],
                        rhs=xts[s][:, d, :], start=(d == 0), stop=(d == DC - 1))
                nc.vector.tensor_copy(dst[:, s * SC:(s + 1) * SC], ps[:])

            def emit_qkv(s):
                emit_xt_dma(s)
                for e in range(3):
                    emit_qkv_part(s, e)

            def emit_vtrans(j):
                b, k = divmod(j, NKC)
                ps = pacc.tile([128, SC], F32, tag="pacc", name="vt_ps")
                pt = ps[:, 0:128]
                nc.tensor.transpose(pt, VT[:, j * 128:(j + 1) * 128], ident[:])
                nc.vector.tensor_copy(vaug[b][0][:, k, 0:64], pt[:, 0:64])
                nc.vector.tensor_copy(vaug[b][1][:, k, 64:128], pt[:, 64:128])

            def emit_finish_stage(st, stage):
                if st is None:
                    return
                if stage == 0:
                    st["invd"] = work.tile([128, QC], F32, tag="invd", name="invd")
                    st["pbc"] = pacc.tile([128, SC], F32, tag="pacc", name="pbc")
                    nc.tensor.matmul(st["pbc"][:], lhsT=inv2[64:128, :],
                                     rhs=st["odA"][64:128, :],
                                     start=True, stop=False)
                    nc.tensor.matmul(st["pbc"][:], lhsT=inv2[0:64, :],
                                     rhs=st["odB"][0:64, :],
                                     start=False, stop=True)
                    nc.vector.reciprocal_approx_fast(st["invd"][:], st["pbc"][:])
                elif stage == 1:
                    st["ot"] = work.tile([128, QC], BF16, tag="ot", name="ot")
                    nc.vector.tensor_mul(out=st["ot"][0:64, :],
                                         in0=st["odA"][0:64, :].bitcast(F32),
                                         in1=st["invd"][0:64, :])
                    nc.vector.tensor_mul(out=st["ot"][64:128, :],
                                         in0=st["odB"][64:128, :].bitcast(F32),
                                         in1=st["invd"][64:128, :])
                else:
                    j = stage - 2
                    osb = work.tile([128, D], F16, tag="osb")
                    for e in range(D // SC):
                        po = pacc.tile([128, SC], F32, tag="pacc", name="po")
                        nc.tensor.matmul(
                            po[:], lhsT=st["ot"][:, j * 128:(j + 1) * 128],
                            rhs=woutT[:, e * SC:(e + 1) * SC],
                            start=True, stop=True)
                        nc.vector.tensor_copy(osb[:, e * SC:(e + 1) * SC], po[:])
                    row = st["q0"] + j * 128
                    nc.sync.dma_start(out_d[row:row + 128, :], osb[:])

            FIN_AT = {4: 0, 6: 1, 8: 2, 10: 3}

            def emit_attn(b, q, prev, filler=None):
                filler = filler or {}
                q0 = b * S + q * QC
                psA = pacc.tile([128, SC], F32, tag="pacc", name="psA")
                psB = pacc.tile([128, SC], F32, tag="pacc", name="psB")
                for k in range(NKC):
                    kcol = b * S + k * 128
                    pss = ps_sc.tile([128, 2 * QC], F32, tag="scores")
                    nc.tensor.matmul(
                        pss[:, 0:QC], lhsT=KT[0:64, kcol:kcol + 128],
                        rhs=QT[0:64, q0:q0 + QC], start=True, stop=True)
                    nc.tensor.matmul(
                        pss[:, QC:2 * QC], lhsT=KT[64:128, kcol:kcol + 128],
                        rhs=QT[64:128, q0:q0 + QC], start=True, stop=True)
                    eb = epool.tile([128, 2 * QC], BF16, tag="eb")
                    nc.scalar.activation(eb[:], pss[:], AF.Exp, scale=float(SCALE))
                    nc.tensor.matmul(psA[:], lhsT=vaug[b][0][:, k, :],
                                     rhs=eb[:, 0:QC],
                                     start=(k == 0), stop=(k == NKC - 1))
                    nc.tensor.matmul(psB[:], lhsT=vaug[b][1][:, k, :],
                                     rhs=eb[:, QC:2 * QC],
                                     start=(k == 0), stop=(k == NKC - 1))
                    if k in FIN_AT:
                        emit_finish_stage(prev, FIN_AT[k])
                    for fn in filler.get(k, ()):
                        fn()
                odA = work.tile([128, QC], F32R, tag="odA", name="odA")
                odB = work.tile([128, QC], F32R, tag="odB", name="odB")
                nc.vector.tensor_copy(odA[:], psA[:])
                nc.vector.tensor_copy(odB[:], psB[:])
                emit_finish_stage(prev, 4)
                emit_finish_stage(prev, 5)
                return {"q0": q0, "odA": odA, "odB": odB}

            for s in range(4):
                emit_qkv(s)
                for j in range(4 * s, 4 * s + 4):
                    emit_vtrans(j)
            prev = None
            for q in range(4):
                prev = emit_attn(0, q, prev)
                s = 4 + q
                emit_qkv(s)
                for j in range(4 * s, 4 * s + 4):
                    emit_vtrans(j)
            for q in range(4):
                prev = emit_attn(1, q, prev)
            for stage in range(4):
                emit_finish_stage(prev, stage)
            emit_finish_stage(prev, 4)
            emit_finish_stage(prev, 5)

    nc.compile()
    return nc


def _get_nc():
    if "nc" not in _cache:
        _cache["nc"] = _build()
    return _cache["nc"]


def _prep_inputs(x, w_qkv, w_out):
    import ml_dtypes
    bf16 = ml_dtypes.bfloat16
    x = np.asarray(x, dtype=np.float32)
    w_qkv = np.asarray(w_qkv, dtype=np.float32)
    w_out = np.asarray(w_out, dtype=np.float32)
    xT = np.ascontiguousarray(x.reshape(BS, D).T.astype(bf16))
    in_maps = []
    for c in range(N_CORES):
        wq = w_qkv[D + 128 * c: D + 128 * (c + 1)]
        wk = w_qkv[2 * D + 128 * c: 2 * D + 128 * (c + 1)]
        wv = w_qkv[128 * c: 128 * (c + 1)]
        wqkvT = np.ascontiguousarray(
            np.concatenate([wq, wk, wv], axis=0).T.astype(bf16))
        woutT = np.ascontiguousarray(
            w_out[:, 128 * c:128 * (c + 1)].T.astype(bf16))
        in_maps.append({"xT": xT, "wqkvT": wqkvT, "woutT": woutT})
    return in_maps


def kernel(x, w_qkv, w_out, b_out):
    from concourse.bass_utils import run_bass_kernel_spmd

    nc = _get_nc()
    in_maps = _prep_inputs(x, w_qkv, w_out)
    b_out = np.asarray(b_out, dtype=np.float32)
    res = run_bass_kernel_spmd(nc, in_maps, core_ids=list(range(N_CORES)))
    acc = np.zeros((BS, D), np.float32)
    for c in range(N_CORES):
        acc += res.results[c]["out"].astype(np.float32)
    acc = acc + b_out[None, :]
    return acc.reshape(B, S, D)
